# revision 1
# baseline (speedup 1.0000x reference)
"""Trainium2 Bass kernel for nn_AttentionBlock (GroupNorm + MHA + proj + residual).

Contract: kernel(**inputs) takes the FULL inputs of reference.setup_inputs()
and returns the FULL (8, 512, 32, 32) output. Internally: data-parallel over
the batch dim across 8 NeuronCores (batch == 8, one image per core); weights
are replicated, so no collectives are needed.

Per-core program (x viewed as (C=512, HW=1024)):
  1. GroupNorm(32 groups): per-channel bn_stats on DVE,group aggregation and
     group->channel broadcast as tiny PE matmuls against constant matrices
     (embedded in the NEFF via inline_tensor).
  2. qkv = qkv_w @ xn as matmuls; weights are PE-transposed once at startup
     (fp32 transpose-mode + ACT-engine PSUM->SBUF copies, pipelined behind
     the 16 weight-tile DMAs issued up front). q,k land in [head_dim, pix]
     layout; v is produced directly TRANSPOSED ([pix, head_dim]) with a ones
     column appended per head.
  3. Attention per head PAIR (two heads share one 128-partition qk chunk):
     the two K=64 score matmuls run on disjoint PE row-groups and stream
     CONCURRENTLY (auto tile_position from base partition). Scores go to two
     live PSUM tiles split by n-half, each exp'd by one ACT op (scale 1/8
     folded into the activation; logits are O(6) so no max subtraction), so
     scores(jc+1) only waits the matching half's exp -- a 2-deep pipeline in
     4 PSUM banks. out_un = v_aug^T @ expS^T accumulates over pixel chunks;
     row 64 of the PSUM result is the softmax denominator for free.
     Normalization: fast-NR reciprocal (DVE) + gpsimd partition_broadcast +
     one DVE multiply, with PSUM freed early via copies so the next pair's
     accumulation and the proj phase are never gated.
  4. proj matmul + bias + residual fused into one scalar_tensor_tensor per
     chunk; proj-weight transposes are emitted right after the last AV so
     the PE stays busy (and HAM-warm) while the last pair normalizes.
All matmuls use float32r (single-pass fp32, ~2 PE cycles/column vs 4 for
exact fp32; measured end-to-end relative error ~3e-5). Producers of matmul
operands write float32r tiles so the BIR verifier rounding rule is met.
"""

import sys
from contextlib import ExitStack

for _p in ("/opt/trn_rl_repo", "/root/.axon_site/_ro/trn_rl_repo"):
    if _p not in sys.path:
        sys.path.append(_p)

import numpy as np

import concourse.bacc as bacc
import concourse.mybir as mybir
import concourse.tile as tile
from concourse.bass_utils import run_bass_kernel_spmd

F32 = mybir.dt.float32
MM_DT = mybir.dt.float32r   # qkv/proj matmul operand dtype
AT_DT = mybir.dt.float32r   # attention matmul operand dtype (q/k/v/expS)

B, C, HW = 8, 512, 1024
GROUPS, HEADS, DH = 32, 8, 64
EPS = 1e-5
KC = C // 128            # 4 channel chunks of 128
OC_QK = 8                # q,k output chunks of 128 (2 heads each)
N_CORES = 8
AF = mybir.ActivationFunctionType
OP = mybir.AluOpType


def _group_mats():
    # A[kc][c, g] = 1/16 if channel (128*kc + c) is in group g  (mean weights)
    # E[kc][g, c] = 1.0  if channel (128*kc + c) is in group g  (broadcast)
    A = np.zeros((KC, 128, GROUPS), np.float32)
    E = np.zeros((KC, GROUPS, 128), np.float32)
    gsz = C // GROUPS  # 16
    for kc in range(KC):
        for c in range(128):
            g = (kc * 128 + c) // gsz
            A[kc, c, g] = 1.0 / gsz
            E[kc, g, c] = 1.0
    return A, E


def _build():
    nc = bacc.Bacc()

    x_h = nc.dram_tensor("x", [C, HW], F32, kind="ExternalInput")
    qkvw_h = nc.dram_tensor("qkv_w", [3 * C, C], F32, kind="ExternalInput")
    qkvb_h = nc.dram_tensor("qkv_b", [3 * C], F32, kind="ExternalInput")
    pw_h = nc.dram_tensor("proj_w", [C, C], F32, kind="ExternalInput")
    pb_h = nc.dram_tensor("proj_b", [C], F32, kind="ExternalInput")
    gnw_h = nc.dram_tensor("gn_w", [C], F32, kind="ExternalInput")
    gnb_h = nc.dram_tensor("gn_b", [C], F32, kind="ExternalInput")
    y_h = nc.dram_tensor("out", [C, HW], F32, kind="ExternalOutput")

    A_np, E_np = _group_mats()
    A_h = nc.inline_tensor(A_np, name="gn_aggr")
    E_h = nc.inline_tensor(E_np, name="gn_bcast")
    I_h = nc.inline_tensor(np.eye(128, dtype=np.float32), name="ident")

    with tile.TileContext(nc) as tc, ExitStack() as ctx:
        per = ctx.enter_context(tc.tile_pool(name="per", bufs=1))
        wstage = ctx.enter_context(tc.tile_pool(name="wstage", bufs=4))
        gwork = ctx.enter_context(tc.tile_pool(name="gwork", bufs=2))
        expp = ctx.enter_context(tc.tile_pool(name="expp", bufs=2))
        recp = ctx.enter_context(tc.tile_pool(name="recp", bufs=2))
        outp = ctx.enter_context(tc.tile_pool(name="outp", bufs=2))
        dramp = ctx.enter_context(tc.tile_pool(name="dramp", bufs=2, space="DRAM"))
        if True:
            # ---------- persistent tiles ----------
            XC = [per.tile([128, HW], F32, name=f"XC{i}", tag=f"XC{i}") for i in range(KC)]
            xn = [per.tile([128, HW], MM_DT, name=f"xn{i}", tag=f"xn{i}") for i in range(KC)]
            wqkT = [per.tile([128, 1024], MM_DT, name=f"wqkT{i}", tag=f"wqkT{i}") for i in range(KC)]
            wvT = [per.tile([128, 512], MM_DT, name=f"wvT{i}", tag=f"wvT{i}") for i in range(KC)]
            pwT = [per.tile([128, 512], MM_DT, name=f"pwT{i}", tag=f"pwT{i}") for i in range(KC)]
            qk = [per.tile([128, HW], AT_DT, name=f"qk{i}", tag=f"qk{i}") for i in range(OC_QK)]
            vt = [per.tile([128, HEADS * (DH + 1)], AT_DT, name=f"vt{i}", tag=f"vt{i}")
                  for i in range(8)]
            att = [per.tile([128, HW], MM_DT, name=f"att{i}", tag=f"att{i}") for i in range(KC)]

            gnw = [per.tile([128, 1], F32, name=f"gnw{i}", tag=f"gnw{i}") for i in range(KC)]
            gnb = [per.tile([128, 1], F32, name=f"gnb{i}", tag=f"gnb{i}") for i in range(KC)]
            qkb = [per.tile([128, 1], F32, name=f"qkb{i}", tag=f"qkb{i}") for i in range(OC_QK)]
            pb = [per.tile([128, 1], F32, name=f"pb{i}", tag=f"pb{i}") for i in range(KC)]
            At = [per.tile([128, GROUPS], F32, name=f"At{i}", tag=f"At{i}") for i in range(KC)]
            Et = [per.tile([GROUPS, 128], F32, name=f"Et{i}", tag=f"Et{i}") for i in range(KC)]
            ident = per.tile([128, 128], F32, name="ident", tag="ident")
            eps_t = per.tile([GROUPS, 1], F32, name="eps", tag="eps")
            bv = per.tile([1, 512], F32, name="bv", tag="bv")
            bvb = per.tile([128, 512], F32, name="bvb", tag="bvb")
            ones8 = per.tile([128, HEADS], F32, name="ones8", tag="ones8")

            # ---------- input DMAs ----------
            # ident first (first transpose needs it); x + W on the two HWDGE
            # queues; small per-chunk tensors via SWDGE so they don't delay
            # the big streams.
            nc.sync.dma_start(out=ident, in_=I_h[:, :])
            # Weight-stage tiles: all 16 DMAs issued up front on the sync
            # queue (5 pool slots keep transfers pipelined ahead of the PE's
            # transposes). x chunks are interleaved between the first W tiles
            # so HBM bandwidth is split between warming the PE and feeding
            # the groupnorm stats chain.
            wfs = [None] * 16

            def _wdma(t):
                wf = wstage.tile([128, 512], F32, name="wstage", tag="wstage")
                srcw = (qkvw_h[t * 128:(t + 1) * 128, :] if t < 12 else
                        pw_h[(t - 12) * 128:(t - 11) * 128, :])
                nc.sync.dma_start(out=wf, in_=srcw)
                wfs[t] = wf

            _wdma(0)
            _wdma(1)
            for kc in range(KC):
                nc.sync.dma_start(out=XC[kc], in_=x_h[kc * 128:(kc + 1) * 128, :])
                _wdma(2 + kc)
            for t in range(6, 16):
                _wdma(t)
            for kc in range(KC):
                nc.gpsimd.dma_start(out=gnw[kc],
                                    in_=gnw_h[kc * 128:(kc + 1) * 128].unsqueeze(-1))
                nc.gpsimd.dma_start(out=gnb[kc],
                                    in_=gnb_h[kc * 128:(kc + 1) * 128].unsqueeze(-1))
                nc.gpsimd.dma_start(out=pb[kc],
                                    in_=pb_h[kc * 128:(kc + 1) * 128].unsqueeze(-1))
                nc.gpsimd.dma_start(out=At[kc], in_=A_h[kc])
                nc.gpsimd.dma_start(out=Et[kc], in_=E_h[kc])
            for oc in range(OC_QK):
                nc.gpsimd.dma_start(out=qkb[oc],
                                    in_=qkvb_h[oc * 128:(oc + 1) * 128].unsqueeze(-1))
            nc.gpsimd.dma_start(out=bv, in_=qkvb_h[1024:1536].unsqueeze(0))
            nc.vector.memset(eps_t, EPS)
            nc.vector.memset(ones8, 1.0)
            nc.sync.dma_start(out=bvb[:],
                              in_=qkvb_h[1024:1536].unsqueeze(0).partition_broadcast(128))

            # ---------- phase A+B: groupnorm, weight transposes, qkv ----------
            with tc.tile_pool(name="ps_tr", bufs=2, space="PSUM") as ps_tr:
                def emit_transposes(ts, trp):
                    for t in ts:
                        wf = wfs[t]
                        for kc in range(KC):
                            tp = trp.tile([128, 128], F32, name="tr", tag="tr")
                            nc.tensor.transpose(
                                tp[:], wf[:, kc * 128:(kc + 1) * 128], ident[:])
                            if t < 8:
                                dst = wqkT[kc][:, t * 128:(t + 1) * 128]
                            elif t < 12:
                                dst = wvT[kc][:, (t - 8) * 128:(t - 7) * 128]
                            else:
                                dst = pwT[kc][:, (t - 12) * 128:(t - 11) * 128]
                            # copy on ACT: DVE is busy with groupnorm stats at
                            # this point, ACT is idle until the attention exps
                            nc.scalar.activation(out=dst, in_=tp[:],
                                                 func=AF.Copy, scale=1.0)

                with tc.tile_pool(name="ps_gn", bufs=1, space="PSUM") as ps_gn, \
                     tc.tile_pool(name="ps_cb", bufs=2, space="PSUM") as ps_cb:
                    gstat = ps_gn.tile([GROUPS, 2], F32, name="gstat", tag="gstat")
                    me = []
                    for kc in range(KC):
                        stats = gwork.tile([128, 2, 6], F32, name="stats", tag="stats")
                        xv = XC[kc][:].rearrange("p (s f) -> p s f", f=512)
                        for s in range(2):
                            nc.vector.bn_stats(out=stats[:, s, :], in_=xv[:, s, :])
                        mv = gwork.tile([128, 2], F32, name="mv", tag="mv")
                        nc.vector.bn_aggr(out=mv, in_=stats)
                        m = gwork.tile([128, 2], F32, name="me", tag="me")
                        nc.vector.tensor_mul(out=m[:, 1:2], in0=mv[:, 0:1], in1=mv[:, 0:1])
                        nc.vector.tensor_add(out=m[:, 1:2], in0=m[:, 1:2], in1=mv[:, 1:2])
                        nc.vector.tensor_copy(out=m[:, 0:1], in_=mv[:, 0:1])
                        me.append(m)

                    emit_transposes(range(0, 8), ps_tr)

                    for kc in range(KC):
                        nc.tensor.matmul(gstat[:], At[kc][:], me[kc][:],
                                         start=(kc == 0), stop=(kc == KC - 1))
                    gs = gwork.tile([GROUPS, 2], F32, name="gs", tag="gs")
                    nc.vector.tensor_copy(out=gs, in_=gstat)
                    var = gwork.tile([GROUPS, 1], F32, name="var", tag="var")
                    nc.vector.tensor_mul(out=var, in0=gs[:, 0:1], in1=gs[:, 0:1])
                    nc.vector.tensor_sub(out=var, in0=gs[:, 1:2], in1=var)
                    srt = gwork.tile([GROUPS, 1], F32, name="srt", tag="srt")
                    nc.scalar.activation(out=srt, in_=var, func=AF.Sqrt,
                                         bias=eps_t[:], scale=1.0)
                    gmr = gwork.tile([GROUPS, 2], F32, name="gmr", tag="gmr")
                    nc.vector.reciprocal(out=gmr[:, 1:2], in_=srt)
                    nc.vector.tensor_copy(out=gmr[:, 0:1], in_=gs[:, 0:1])

                    for kc in range(KC):
                        cb = ps_cb.tile([128, 2], F32, name="cb", tag="cb")
                        nc.tensor.matmul(cb[:], Et[kc][:], gmr[:], start=True, stop=True)
                        cbs = gwork.tile([128, 2], F32, name="cbs", tag="cbs")
                        nc.vector.tensor_copy(out=cbs, in_=cb)
                        sc = gwork.tile([128, 1], F32, name=f"sc{kc}", tag=f"sc{kc}")
                        sh = gwork.tile([128, 1], F32, name=f"sh{kc}", tag=f"sh{kc}")
                        nc.vector.tensor_mul(out=sc, in0=cbs[:, 1:2], in1=gnw[kc][:])
                        nc.vector.tensor_mul(out=sh, in0=cbs[:, 0:1], in1=sc)
                        nc.vector.tensor_sub(out=sh, in0=gnb[kc][:], in1=sh)
                        nc.vector.tensor_scalar(out=xn[kc][:], in0=XC[kc][:],
                                                scalar1=sc[:], scalar2=sh[:],
                                                op0=OP.mult, op1=OP.add)

                with tc.tile_pool(name="ps_qk", bufs=2, space="PSUM") as ps_qk, \
                     tc.tile_pool(name="ps_v", bufs=2, space="PSUM") as ps_v:
                    for oc in range(OC_QK):
                        pq = ps_qk.tile([128, HW], F32, name="pq", tag="pq")
                        for kc in range(KC):
                            for nh in range(2):
                                nc.tensor.matmul(
                                    pq[:, nh * 512:(nh + 1) * 512],
                                    wqkT[kc][:, oc * 128:(oc + 1) * 128],
                                    xn[kc][:, nh * 512:(nh + 1) * 512],
                                    start=(kc == 0), stop=(kc == KC - 1),
                                    skip_group_check=True)
                        nc.vector.tensor_scalar(out=qk[oc][:], in0=pq[:],
                                                scalar1=qkb[oc][:], scalar2=None,
                                                op0=OP.add)
                        if oc == 0:
                            emit_transposes(range(8, 12), ps_tr)
                    for jc in range(8):
                        pv = ps_v.tile([128, 512], F32, name="pv", tag="pv")
                        for kc in range(KC):
                            nc.tensor.matmul(pv[:],
                                             xn[kc][:, jc * 128:(jc + 1) * 128],
                                             wvT[kc][:],
                                             start=(kc == 0), stop=(kc == KC - 1))
                        vt3d = vt[jc][:].rearrange("p (h e) -> p h e", h=HEADS)
                        nc.vector.tensor_copy(out=vt3d[:, :, DH:DH + 1],
                                              in_=ones8[:].unsqueeze(-1))
                        nc.vector.tensor_add(
                            out=vt3d[:, :, 0:DH],
                            in0=pv[:].rearrange("p (h d) -> p h d", h=HEADS),
                            in1=bvb[:].rearrange("p (h d) -> p h d", h=HEADS))

            # ---------- phase C: attention ----------
            # Heads are processed in pairs (2p, 2p+1): their q/k live in the
            # upper/lower 64 partitions of one qk chunk, so the two K=64
            # score matmuls land on disjoint PE row-groups (tile_position
            # auto-derived from base partition) and stream CONCURRENTLY
            # through separate XBUSes -- 2x effective PE throughput for
            # scores. Both heads' scores share one [128, 2048] PSUM tile
            # (4 banks) so a single exp covers the pair.
            with tc.tile_pool(name="ps_s", bufs=1, space="PSUM") as ps_s, \
                 tc.tile_pool(name="ps_av", bufs=1, space="PSUM") as ps_av:
                for pr in range(HEADS // 2):
                    qt, kt = qk[pr], qk[4 + pr]
                    pavA = ps_av.tile([DH + 1, HW], F32, name="pavA", tag="pavA")
                    pavB = ps_av.tile([DH + 1, HW], F32, name="pavB", tag="pavB")
                    exs = [None] * 8

                    def emit_av(jc):
                        ex = exs[jc]
                        for t, pav in ((0, pavA), (1, pavB)):
                            h = 2 * pr + t
                            for nh in range(2):
                                nc.tensor.matmul(
                                    pav[:, nh * 512:(nh + 1) * 512],
                                    vt[jc][:, h * (DH + 1):(h + 1) * (DH + 1)],
                                    ex[:, t * HW + nh * 512:
                                       t * HW + (nh + 1) * 512],
                                    start=(jc == 0), stop=(jc == 7),
                                    skip_group_check=True)

                    # software-pipelined emission: scores(jc) | AV(jc-1) |
                    # exp(jc). The two exps are split by n-half (strided
                    # across both heads' halves) so each covers one PSUM bank
                    # per head -- scores(jc+1)'s nh0 pair only waits on
                    # exp_nh0(jc), halving the exp-latency in the PE's
                    # critical path.
                    # Two live score tiles, one per n-half: [A-half | B-half].
                    # Each is exp'd by one ACT op, so the nh0 tile is free for
                    # scores(jc+1) while exp of the nh1 tile still runs --
                    # a 2-deep pipeline within 4 PSUM banks.
                    pss = [ps_s.tile([128, HW], F32, name=f"pss{i}",
                                     tag=f"pss{i}") for i in range(2)]
                    for jc in range(8):
                        for nh in range(2):
                            for t in range(2):
                                nc.tensor.matmul(
                                    pss[nh][:, t * 512:(t + 1) * 512],
                                    kt[64 * t:64 * t + DH,
                                       jc * 128:(jc + 1) * 128],
                                    qt[64 * t:64 * t + DH,
                                       nh * 512:(nh + 1) * 512],
                                    start=True, stop=True)
                        if jc > 0:
                            emit_av(jc - 1)
                        ex = expp.tile([128, 2 * HW], AT_DT, name="expT",
                                       tag="expT")
                        ex3 = ex[:].rearrange("p (t m) -> p t m", t=2)
                        for nh in range(2):
                            nc.scalar.activation(
                                out=ex3[:, :, nh * 512:(nh + 1) * 512],
                                in_=pss[nh][:].rearrange("p (t n) -> p t n",
                                                         t=2),
                                func=AF.Exp, scale=float(DH) ** -0.5)
                        exs[jc] = ex
                    emit_av(7)
                    last = (pr == HEADS // 2 - 1)
                    dens, avss = [], []
                    for t, pav in ((0, pavA), (1, pavB)):
                        # denominator rows first: the reciprocals (and the
                        # gpsimd broadcasts behind them) start as early as
                        # possible; the accumulator copies then free the
                        # PSUM banks for the next pair.
                        den = recp.tile([1, HW], F32, name="den", tag="den", bufs=2)
                        nc.vector.tensor_copy(out=den[:], in_=pav[DH:DH + 1, :])
                        dens.append(den)
                    for t, pav in ((0, pavA), (1, pavB)):
                        avs = recp.tile([DH, HW], F32, name="avs", tag="avs",
                                        bufs=3)
                        nc.vector.tensor_copy(out=avs[:], in_=pav[0:DH, :])
                        avss.append(avs)
                    for t in range(2):
                        rec = recp.tile([1, HW], F32, name="rec", tag="rec", bufs=2)
                        rb = recp.tile([DH, HW], F32, name="rb", tag="rb", bufs=2)
                        nc.vector.reciprocal_approx_fast(out=rec[:], in_=dens[t][:])
                        nc.gpsimd.partition_broadcast(out_ap=rb[:], in_ap=rec[:])
                        halves = (slice(0, 512), slice(512, HW)) if last \
                            else (slice(0, HW),)
                        for sl in halves:
                            nc.vector.tensor_mul(
                                out=att[pr][64 * t:64 * t + DH, sl],
                                in0=avss[t][:, sl], in1=rb[:, sl])

            # ---------- phase D: proj + bias + residual ----------
            with tc.tile_pool(name="ps_tr2", bufs=2, space="PSUM") as ps_tr2, \
                 tc.tile_pool(name="ps_p", bufs=4, space="PSUM") as ps_p:
                # proj-weight transposes land here: PE work that fills the
                # last softmax-normalize latency and keeps HAM warm for proj
                emit_transposes(range(12, 16), ps_tr2)
                for oc in range(KC):
                    pp = ps_p.tile([128, HW], F32, name="pp", tag="pp", bufs=2)
                    ot = outp.tile([128, HW], F32, name="ot", tag="ot")
                    for kc in range(KC):
                        for nh in range(2):
                            nc.tensor.matmul(
                                pp[:, nh * 512:(nh + 1) * 512],
                                pwT[kc][:, oc * 128:(oc + 1) * 128],
                                att[kc][:, nh * 512:(nh + 1) * 512],
                                start=(kc == 0), stop=(kc == KC - 1),
                                skip_group_check=True)
                    nc.vector.scalar_tensor_tensor(out=ot[:], in0=pp[:],
                                                   scalar=pb[oc][:],
                                                   in1=XC[oc][:],
                                                   op0=OP.add, op1=OP.add)
                    nc.sync.dma_start(out=y_h[oc * 128:(oc + 1) * 128, :],
                                      in_=ot[:])
    nc.compile()
    return nc


_NC = None


def _get_nc():
    global _NC
    if _NC is None:
        _NC = _build()
    return _NC


def _run(inputs, **kwargs):
    nc = _get_nc()
    x = np.ascontiguousarray(np.asarray(inputs["x"], dtype=np.float32))
    shared = {
        "qkv_w": np.ascontiguousarray(np.asarray(inputs["qkv_w"], np.float32)),
        "qkv_b": np.ascontiguousarray(np.asarray(inputs["qkv_b"], np.float32)),
        "proj_w": np.ascontiguousarray(np.asarray(inputs["proj_w"], np.float32)),
        "proj_b": np.ascontiguousarray(np.asarray(inputs["proj_b"], np.float32)),
        "gn_w": np.ascontiguousarray(np.asarray(inputs["gn_w"], np.float32)),
        "gn_b": np.ascontiguousarray(np.asarray(inputs["gn_b"], np.float32)),
    }
    in_maps = [dict(shared, x=x[m].reshape(C, HW)) for m in range(B)]
    res = run_bass_kernel_spmd(nc, in_maps, core_ids=list(range(N_CORES)), **kwargs)
    out = np.stack([res.results[m]["out"] for m in range(B)])
    return out.reshape(B, C, 32, 32).astype(np.float32), res


def kernel(**inputs):
    out, _ = _run(inputs)
    return out



# revision 10
# speedup vs baseline: 1.2216x; 1.2216x over previous
"""Trainium2 Bass kernel for nn_AttentionBlock (GroupNorm + MHA + proj + residual).

Contract: kernel(**inputs) takes the FULL inputs of reference.setup_inputs()
and returns the FULL (8, 512, 32, 32) output. Internally: data-parallel over
the batch dim across 8 NeuronCores (batch == 8, one image per core); weights
are replicated, so no collectives are needed.

v2 design (vs the fp32r baseline):
  * All matmul operands are bf16 (1 PE cycle/column vs 2 for fp32r); PSUM
    accumulation stays fp32. Measured end-to-end error stays well under the
    2e-2 gate.
  * qkv_w / proj_w are transposed and cast to bf16 ON THE HOST (numpy) and
    fed pre-transposed via DRAM. This removes all 64 PE transpose ops, their
    64 ACT PSUM->SBUF copies, and halves the weight DMA bytes.
  * The softmax exp stream on the ACT engine is the hard floor (8.4M
    elements ~= 64 ops x ~1us @1.2GHz, dtype-independent). The attention
    loop is restructured so ACT is saturated while the PE fills its slack
    with the REMAINING qkv/v matmuls:
      - heads processed in pairs (2 heads share one 128-partition q/k chunk;
        their K=64 score matmuls run concurrently on disjoint PE row groups)
      - each pair is processed in two i-halves of 512 pixels; per (pair,
        half, jc): scores -> one contiguous [128,1024] exp op -> AV.
      - PSUM: pss 2x[128,1024] (4 banks) + pav [65,1024] (2 banks) + pq
        2x[128,512] (2 banks) = exactly 8 banks, which is what lets qkv
        chunks stream during attention.
  * v is produced transposed ([pix, dh]) with a ones column per head, so the
    AV matmul's 65th row accumulates the softmax denominator for free.
  * ACT tables are prewarmed (dummy sqrt at t=0, dummy exp right after the
    real sqrt) so neither ~2.7us ACT_TABLE_LOAD sits on the critical path.
"""

import sys
from contextlib import ExitStack

for _p in ("/opt/trn_rl_repo", "/root/.axon_site/_ro/trn_rl_repo"):
    if _p not in sys.path:
        sys.path.append(_p)

import numpy as np
import ml_dtypes

import concourse.bacc as bacc
import concourse.mybir as mybir
import concourse.tile as tile
from concourse.bass_utils import run_bass_kernel_spmd

F32 = mybir.dt.float32
BF16 = mybir.dt.bfloat16

B, C, HW = 8, 512, 1024
GROUPS, HEADS, DH = 32, 8, 64
EPS = 1e-5
KC = C // 128            # 4 channel chunks of 128
N_CORES = 8
AF = mybir.ActivationFunctionType
OP = mybir.AluOpType


def _group_mats():
    # A[kc][c, g] = 1/16 if channel (128*kc + c) is in group g  (mean weights)
    # E[kc][g, c] = 1.0  if channel (128*kc + c) is in group g  (broadcast)
    A = np.zeros((KC, 128, GROUPS), np.float32)
    E = np.zeros((KC, GROUPS, 128), np.float32)
    gsz = C // GROUPS  # 16
    for kc in range(KC):
        for c in range(128):
            g = (kc * 128 + c) // gsz
            A[kc, c, g] = 1.0 / gsz
            E[kc, g, c] = 1.0
    return A, E


def _build():
    nc = bacc.Bacc()

    x_h = nc.dram_tensor("x", [C, HW], F32, kind="ExternalInput")
    # host-pretransposed, bf16: wqkT[c, o] covers q (o 0:512) and k (512:1024)
    wqkT_h = nc.dram_tensor("wqkT", [C, 2 * C], BF16, kind="ExternalInput")
    wvT_h = nc.dram_tensor("wvT", [C, C], BF16, kind="ExternalInput")
    pwT_h = nc.dram_tensor("pwT", [C, C], BF16, kind="ExternalInput")
    qkvb_h = nc.dram_tensor("qkv_b", [3 * C], F32, kind="ExternalInput")
    pb_h = nc.dram_tensor("proj_b", [C], F32, kind="ExternalInput")
    gnw_h = nc.dram_tensor("gn_w", [C], F32, kind="ExternalInput")
    gnb_h = nc.dram_tensor("gn_b", [C], F32, kind="ExternalInput")
    y_h = nc.dram_tensor("out", [C, HW], F32, kind="ExternalOutput")

    A_np, E_np = _group_mats()
    A_h = nc.inline_tensor(A_np, name="gn_aggr")
    E_h = nc.inline_tensor(E_np, name="gn_bcast")

    with tile.TileContext(nc) as tc, ExitStack() as ctx:
        per = ctx.enter_context(tc.tile_pool(name="per", bufs=1))
        gwork = ctx.enter_context(tc.tile_pool(name="gwork", bufs=2))
        expp = ctx.enter_context(tc.tile_pool(name="expp", bufs=6))
        recp = ctx.enter_context(tc.tile_pool(name="recp", bufs=2))
        outp = ctx.enter_context(tc.tile_pool(name="outp", bufs=2))

        # ---------- persistent tiles ----------
        XC = [per.tile([128, HW], F32, name=f"XC{i}", tag=f"XC{i}") for i in range(KC)]
        xn = [per.tile([128, HW], BF16, name=f"xn{i}", tag=f"xn{i}") for i in range(KC)]
        wqkT = [per.tile([128, 1024], BF16, name=f"wqkT{i}", tag=f"wqkT{i}") for i in range(KC)]
        wvT = [per.tile([128, 512], BF16, name=f"wvT{i}", tag=f"wvT{i}") for i in range(KC)]
        pwT = [per.tile([128, 512], BF16, name=f"pwT{i}", tag=f"pwT{i}") for i in range(KC)]
        qk = [per.tile([128, HW], BF16, name=f"qk{i}", tag=f"qk{i}") for i in range(8)]
        vt = [per.tile([128, HEADS * (DH + 1)], BF16, name=f"vt{i}", tag=f"vt{i}")
              for i in range(8)]
        att = [per.tile([128, HW], BF16, name=f"att{i}", tag=f"att{i}") for i in range(KC)]

        gnw = [per.tile([128, 1], F32, name=f"gnw{i}", tag=f"gnw{i}") for i in range(KC)]
        gnb = [per.tile([128, 1], F32, name=f"gnb{i}", tag=f"gnb{i}") for i in range(KC)]
        qkb = [per.tile([128, 1], F32, name=f"qkb{i}", tag=f"qkb{i}") for i in range(8)]
        pb = [per.tile([128, 1], F32, name=f"pb{i}", tag=f"pb{i}") for i in range(KC)]
        At = [per.tile([128, GROUPS], F32, name=f"At{i}", tag=f"At{i}") for i in range(KC)]
        Et = [per.tile([GROUPS, 128], F32, name=f"Et{i}", tag=f"Et{i}") for i in range(KC)]
        eps_t = per.tile([GROUPS, 1], F32, name="eps", tag="eps")
        bvb = per.tile([128, 512], F32, name="bvb", tag="bvb")
        ones8 = per.tile([128, HEADS], F32, name="ones8", tag="ones8")
        warm = per.tile([1, 1], F32, name="warm", tag="warm")

        # ---------- input DMAs ----------
        # x + the q/k weights on the sync HWDGE queue (x first: groupnorm
        # stats are the head of the dependency chain); everything else via
        # the gpsimd SWDGE so the two streams overlap.
        for kc in range(KC):
            nc.sync.dma_start(out=XC[kc], in_=x_h[kc * 128:(kc + 1) * 128, :])
        for kc in range(KC):
            nc.sync.dma_start(out=wqkT[kc], in_=wqkT_h[kc * 128:(kc + 1) * 128, :])
        for kc in range(KC):
            nc.gpsimd.dma_start(out=gnw[kc],
                                in_=gnw_h[kc * 128:(kc + 1) * 128].unsqueeze(-1))
            nc.gpsimd.dma_start(out=gnb[kc],
                                in_=gnb_h[kc * 128:(kc + 1) * 128].unsqueeze(-1))
            nc.gpsimd.dma_start(out=pb[kc],
                                in_=pb_h[kc * 128:(kc + 1) * 128].unsqueeze(-1))
            nc.gpsimd.dma_start(out=At[kc], in_=A_h[kc])
            nc.gpsimd.dma_start(out=Et[kc], in_=E_h[kc])
        for oc in range(8):
            nc.gpsimd.dma_start(out=qkb[oc],
                                in_=qkvb_h[oc * 128:(oc + 1) * 128].unsqueeze(-1))
        nc.gpsimd.dma_start(out=bvb[:],
                            in_=qkvb_h[1024:1536].unsqueeze(0).partition_broadcast(128))
        for kc in range(KC):
            nc.gpsimd.dma_start(out=wvT[kc], in_=wvT_h[kc * 128:(kc + 1) * 128, :])
        for kc in range(KC):
            nc.gpsimd.dma_start(out=pwT[kc], in_=pwT_h[kc * 128:(kc + 1) * 128, :])

        nc.vector.memset(eps_t, EPS)
        nc.vector.memset(ones8, 1.0)
        nc.vector.memset(warm, 1.0)
        # prewarm the SQRT table set while DMAs stream (first use ~t=8us)
        nc.scalar.activation(out=warm[:], in_=warm[:], func=AF.Sqrt, scale=1.0)

        # ---------- groupnorm ----------
        with tc.tile_pool(name="ps_gn", bufs=1, space="PSUM") as ps_gn, \
             tc.tile_pool(name="ps_cb", bufs=2, space="PSUM") as ps_cb:
            gstat = ps_gn.tile([GROUPS, 2], F32, name="gstat", tag="gstat")
            me = []
            for kc in range(KC):
                stats = gwork.tile([128, 2, 6], F32, name="stats", tag="stats")
                xv = XC[kc][:].rearrange("p (s f) -> p s f", f=512)
                for s in range(2):
                    nc.vector.bn_stats(out=stats[:, s, :], in_=xv[:, s, :])
                mv = gwork.tile([128, 2], F32, name="mv", tag="mv")
                nc.vector.bn_aggr(out=mv, in_=stats)
                m = gwork.tile([128, 2], F32, name="me", tag="me")
                nc.vector.tensor_mul(out=m[:, 1:2], in0=mv[:, 0:1], in1=mv[:, 0:1])
                nc.vector.tensor_add(out=m[:, 1:2], in0=m[:, 1:2], in1=mv[:, 1:2])
                nc.vector.tensor_copy(out=m[:, 0:1], in_=mv[:, 0:1])
                me.append(m)

            for kc in range(KC):
                nc.tensor.matmul(gstat[:], At[kc][:], me[kc][:],
                                 start=(kc == 0), stop=(kc == KC - 1))
            gs = gwork.tile([GROUPS, 2], F32, name="gs", tag="gs")
            nc.vector.tensor_copy(out=gs, in_=gstat)
            var = gwork.tile([GROUPS, 1], F32, name="var", tag="var")
            nc.vector.tensor_mul(out=var, in0=gs[:, 0:1], in1=gs[:, 0:1])
            nc.vector.tensor_sub(out=var, in0=gs[:, 1:2], in1=var)
            srt = gwork.tile([GROUPS, 1], F32, name="srt", tag="srt")
            nc.scalar.activation(out=srt, in_=var, func=AF.Sqrt,
                                 bias=eps_t[:], scale=1.0)
            # prewarm the EXP table set (loads during the qkv phase)
            nc.scalar.activation(out=warm[:], in_=warm[:], func=AF.Exp, scale=1.0)
            gmr = gwork.tile([GROUPS, 2], F32, name="gmr", tag="gmr")
            nc.vector.reciprocal(out=gmr[:, 1:2], in_=srt)
            nc.vector.tensor_copy(out=gmr[:, 0:1], in_=gs[:, 0:1])

            for kc in range(KC):
                cb = ps_cb.tile([128, 2], F32, name="cb", tag="cb")
                nc.tensor.matmul(cb[:], Et[kc][:], gmr[:], start=True, stop=True)
                cbs = gwork.tile([128, 2], F32, name="cbs", tag="cbs")
                nc.vector.tensor_copy(out=cbs, in_=cb)
                sc = gwork.tile([128, 1], F32, name=f"sc{kc}", tag=f"sc{kc}")
                sh = gwork.tile([128, 1], F32, name=f"sh{kc}", tag=f"sh{kc}")
                nc.vector.tensor_mul(out=sc, in0=cbs[:, 1:2], in1=gnw[kc][:])
                nc.vector.tensor_mul(out=sh, in0=cbs[:, 0:1], in1=sc)
                nc.vector.tensor_sub(out=sh, in0=gnb[kc][:], in1=sh)
                nc.vector.tensor_scalar(out=xn[kc][:], in0=XC[kc][:],
                                        scalar1=sc[:], scalar2=sh[:],
                                        op0=OP.mult, op1=OP.add)

        # ---------- qkv / attention / proj ----------
        # pq: 2 bufs x [128,512] (2 banks) -- qkv + proj chunks
        # pss: 2 bufs x [128,1024] (4 banks) -- scores for one (pair,half,jc)
        # pav: 1 buf x [65,1024] (2 banks) -- AV accumulator for one half
        with tc.tile_pool(name="ps_q", bufs=2, space="PSUM") as ps_q, \
             tc.tile_pool(name="ps_s", bufs=2, space="PSUM") as ps_s, \
             tc.tile_pool(name="ps_av", bufs=1, space="PSUM") as ps_av:

            def emit_qk_half(oc, nh):
                # q/k output chunk oc (128 rows = 2 heads), pixel half nh
                pq = ps_q.tile([128, 512], F32, name="pq", tag="pq")
                for kc in range(KC):
                    nc.tensor.matmul(
                        pq[:],
                        wqkT[kc][:, oc * 128:(oc + 1) * 128],
                        xn[kc][:, nh * 512:(nh + 1) * 512],
                        start=(kc == 0), stop=(kc == KC - 1),
                        skip_group_check=True)
                nc.vector.tensor_scalar(out=qk[oc][:, nh * 512:(nh + 1) * 512],
                                        in0=pq[:], scalar1=qkb[oc][:],
                                        scalar2=None, op0=OP.add)

            def emit_vt(jc):
                pv = ps_q.tile([128, 512], F32, name="pv", tag="pq")
                for kc in range(KC):
                    nc.tensor.matmul(pv[:],
                                     xn[kc][:, jc * 128:(jc + 1) * 128],
                                     wvT[kc][:],
                                     start=(kc == 0), stop=(kc == KC - 1))
                vt3 = vt[jc][:].rearrange("p (h e) -> p h e", h=HEADS)
                nc.vector.tensor_copy(out=vt3[:, :, DH:DH + 1],
                                      in_=ones8[:].unsqueeze(-1))
                nc.vector.tensor_add(
                    out=vt3[:, :, 0:DH],
                    in0=pv[:].rearrange("p (h d) -> p h d", h=HEADS),
                    in1=bvb[:].rearrange("p (h d) -> p h d", h=HEADS))

            def emit_proj_half(oc, nh):
                pp = ps_q.tile([128, 512], F32, name="pp", tag="pq")
                ot = outp.tile([128, 512], F32, name="ot", tag="ot")
                for kc in range(KC):
                    nc.tensor.matmul(
                        pp[:],
                        pwT[kc][:, oc * 128:(oc + 1) * 128],
                        att[kc][:, nh * 512:(nh + 1) * 512],
                        start=(kc == 0), stop=(kc == KC - 1),
                        skip_group_check=True)
                nc.vector.scalar_tensor_tensor(
                    out=ot[:], in0=pp[:], scalar=pb[oc][:],
                    in1=XC[oc][:, nh * 512:(nh + 1) * 512],
                    op0=OP.add, op1=OP.add)
                nc.sync.dma_start(
                    out=y_h[oc * 128:(oc + 1) * 128, nh * 512:(nh + 1) * 512],
                    in_=ot[:])

            # filler units: PE work to interleave into the attention phase,
            # highest-priority first. v tiles 2..7 must land before pair 0
            # consumes them (emitted inside window 0); later pairs' q/k
            # chunks follow.
            filler = []
            for oc in (1, 5, 2, 6, 3, 7):
                for nh in range(2):
                    filler.append((emit_qk_half, oc, nh))
            fill_i = 0

            def emit_filler(n):
                nonlocal fill_i
                for _ in range(n):
                    if fill_i < len(filler):
                        f = filler[fill_i]
                        f[0](*f[1:])
                        fill_i += 1

            # upfront: q/k for pair 0 and all v tiles (v is consumed by the
            # very first AVs; building it as window-0 filler would starve
            # the exp stream)
            for nh in range(2):
                emit_qk_half(0, nh)
            for nh in range(2):
                emit_qk_half(4, nh)
            for jc in range(8):
                emit_vt(jc)

            for pr in range(HEADS // 2):
                qt, kt = qk[pr], qk[4 + pr]
                for hf in range(2):
                    pav = ps_av.tile([DH + 1, HW], F32, name="pav", tag="pav")
                    exs = [None] * 8
                    for jc in range(8):
                        pss = ps_s.tile([128, HW], F32, name="pss", tag="pss")
                        for t in range(2):
                            nc.tensor.matmul(
                                pss[:, t * 512:(t + 1) * 512],
                                kt[64 * t:64 * t + DH, jc * 128:(jc + 1) * 128],
                                qt[64 * t:64 * t + DH,
                                   hf * 512:(hf + 1) * 512],
                                start=True, stop=True)
                        ex = expp.tile([128, HW], BF16, name="expT", tag="expT")
                        nc.scalar.activation(out=ex[:], in_=pss[:],
                                             func=AF.Exp,
                                             scale=float(DH) ** -0.5)
                        exs[jc] = ex
                        if jc > 0:
                            for t in range(2):
                                h = 2 * pr + t
                                nc.tensor.matmul(
                                    pav[:, t * 512:(t + 1) * 512],
                                    vt[jc - 1][:, h * (DH + 1):(h + 1) * (DH + 1)],
                                    exs[jc - 1][:, t * 512:(t + 1) * 512],
                                    start=(jc - 1 == 0), stop=False,
                                    skip_group_check=True)
                        if jc in (2, 5):
                            emit_filler(1)
                    for t in range(2):
                        h = 2 * pr + t
                        nc.tensor.matmul(
                            pav[:, t * 512:(t + 1) * 512],
                            vt[7][:, h * (DH + 1):(h + 1) * (DH + 1)],
                            exs[7][:, t * 512:(t + 1) * 512],
                            start=False, stop=True,
                            skip_group_check=True)
                    # softmax normalize: row 64 of each half of pav is the
                    # denominator. reciprocal on DVE, broadcast on gpsimd,
                    # multiply straight out of PSUM.
                    den = recp.tile([1, HW], F32, name="den", tag="den")
                    nc.vector.tensor_copy(out=den[:], in_=pav[DH:DH + 1, :])
                    recs, rbs = [], []
                    for t in range(2):
                        rc = recp.tile([1, 512], F32, name=f"rec{t}",
                                       tag=f"rec{t}")
                        nc.vector.reciprocal_approx_fast(
                            out=rc[:], in_=den[0:1, t * 512:(t + 1) * 512])
                        recs.append(rc)
                    for t in range(2):
                        rb = recp.tile([DH, 512], F32, name=f"rb{t}",
                                       tag=f"rb{t}")
                        nc.gpsimd.partition_broadcast(out_ap=rb[:],
                                                      in_ap=recs[t][:])
                        rbs.append(rb)
                    for t in range(2):
                        nc.vector.tensor_mul(
                            out=att[pr][64 * t:64 * t + DH,
                                        hf * 512:(hf + 1) * 512],
                            in0=pav[0:DH, t * 512:(t + 1) * 512],
                            in1=rbs[t][:])

            # ---------- proj + bias + residual ----------
            for oc in range(KC):
                for nh in range(2):
                    emit_proj_half(oc, nh)
    nc.compile()
    return nc


_NC = None


def _get_nc():
    global _NC
    if _NC is None:
        _NC = _build()
    return _NC


def _run(inputs, **kwargs):
    nc = _get_nc()
    x = np.ascontiguousarray(np.asarray(inputs["x"], dtype=np.float32))
    qkv_w = np.asarray(inputs["qkv_w"], np.float32)
    proj_w = np.asarray(inputs["proj_w"], np.float32)
    shared = {
        "wqkT": np.ascontiguousarray(qkv_w[0:1024].T).astype(ml_dtypes.bfloat16),
        "wvT": np.ascontiguousarray(qkv_w[1024:1536].T).astype(ml_dtypes.bfloat16),
        "pwT": np.ascontiguousarray(proj_w.T).astype(ml_dtypes.bfloat16),
        "qkv_b": np.ascontiguousarray(np.asarray(inputs["qkv_b"], np.float32)),
        "proj_b": np.ascontiguousarray(np.asarray(inputs["proj_b"], np.float32)),
        "gn_w": np.ascontiguousarray(np.asarray(inputs["gn_w"], np.float32)),
        "gn_b": np.ascontiguousarray(np.asarray(inputs["gn_b"], np.float32)),
    }
    in_maps = [dict(shared, x=x[m].reshape(C, HW)) for m in range(B)]
    res = run_bass_kernel_spmd(nc, in_maps, core_ids=list(range(N_CORES)), **kwargs)
    out = np.stack([res.results[m]["out"] for m in range(B)])
    return out.reshape(B, C, 32, 32).astype(np.float32), res


def kernel(**inputs):
    out, _ = _run(inputs)
    return out


# revision 19
# speedup vs baseline: 1.3547x; 1.1089x over previous
"""Trainium2 Bass kernel for nn_AttentionBlock (GroupNorm + MHA + proj + residual).

Contract: kernel(**inputs) takes the FULL inputs of reference.setup_inputs()
and returns the FULL (8, 512, 32, 32) output. Internally: data-parallel over
the batch dim across 8 NeuronCores (batch == 8, one image per core); weights
are replicated, so no collectives are needed.

Design notes (v4):
  * All matmul operands are bf16 (1 PE cycle/column); PSUM stays fp32.
  * qkv_w / proj_w are transposed and cast to bf16 ON THE HOST and fed
    pre-transposed via DRAM: no PE transposes, no ACT copies, half the DMA.
  * The ACT-engine softmax exp stream is the kernel's spine (64 ops x
    ~1.1us, dtype-independent rate). Everything is scheduled to keep it
    saturated:
      - attention runs per (head-pair, pixel-half) window; per jc chunk:
        2 concurrent K=64 score matmuls -> one contiguous [128,1024] exp ->
        2 AV matmuls lagging one chunk.
      - AV accumulators are two 1-bank [65,512] PSUM tiles per window
        (row 64 = softmax denominator via a ones column in v), rotated
        through 3 pool slots so the normalize chain of window w never
        stalls window w+1.
      - PSUM budget: scores 2x[128,1024] (4 banks) + pav 3x[65,512]
        (3 banks) + pq [128,512] (1 bank) = 8 banks exactly.
      - the pq bank lets leftover qkv matmuls stream through the attention
        phase as single-matmul filler (1 per exp slot), and lets the nh=0
        half of proj run inside the last window.
  * Startup: x is DMA'd on two queues, small tensors are consolidated into
    a few strided DMAs, ACT tables (sqrt, exp) are prewarmed off-path.
"""

import sys
from contextlib import ExitStack

for _p in ("/opt/trn_rl_repo", "/root/.axon_site/_ro/trn_rl_repo"):
    if _p not in sys.path:
        sys.path.append(_p)

import numpy as np
import ml_dtypes

import concourse.bacc as bacc
import concourse.mybir as mybir
import concourse.tile as tile
from concourse.bass_utils import run_bass_kernel_spmd

F32 = mybir.dt.float32
BF16 = mybir.dt.bfloat16

B, C, HW = 8, 512, 1024
GROUPS, HEADS, DH = 32, 8, 64
EPS = 1e-5
KC = C // 128            # 4 channel chunks of 128
N_CORES = 8
AF = mybir.ActivationFunctionType
OP = mybir.AluOpType


def _group_mats():
    # A[kc][c, g] = 1/16 if channel (128*kc + c) is in group g  (mean weights)
    # E[kc][g, c] = 1.0  if channel (128*kc + c) is in group g  (broadcast)
    A = np.zeros((KC, 128, GROUPS), np.float32)
    E = np.zeros((KC, GROUPS, 128), np.float32)
    gsz = C // GROUPS  # 16
    for kc in range(KC):
        for c in range(128):
            g = (kc * 128 + c) // gsz
            A[kc, c, g] = 1.0 / gsz
            E[kc, g, c] = 1.0
    return A, E


def _build():
    nc = bacc.Bacc()

    x_h = nc.dram_tensor("x", [C, HW], F32, kind="ExternalInput")
    # host-pretransposed, bf16: wqkT[c, o] covers q (o 0:512) and k (512:1024)
    wqkT_h = nc.dram_tensor("wqkT", [C, 2 * C], BF16, kind="ExternalInput")
    wvT_h = nc.dram_tensor("wvT", [C, C], BF16, kind="ExternalInput")
    pwT_h = nc.dram_tensor("pwT", [C, C], BF16, kind="ExternalInput")
    qkvb_h = nc.dram_tensor("qkv_b", [3 * C], F32, kind="ExternalInput")
    pb_h = nc.dram_tensor("proj_b", [C], F32, kind="ExternalInput")
    gnw_h = nc.dram_tensor("gn_w", [C], F32, kind="ExternalInput")
    gnb_h = nc.dram_tensor("gn_b", [C], F32, kind="ExternalInput")
    y_h = nc.dram_tensor("out", [C, HW], F32, kind="ExternalOutput")
    import os as _os
    DBG = bool(_os.environ.get("KDBG"))
    if DBG:
        dbg_xn = nc.dram_tensor("dbg_xn", [C, HW], F32, kind="ExternalOutput")
        dbg_qk = nc.dram_tensor("dbg_qk", [8, 128, HW], F32, kind="ExternalOutput")
        dbg_vt = nc.dram_tensor("dbg_vt", [8, 128, 520], F32, kind="ExternalOutput")
        dbg_att = nc.dram_tensor("dbg_att", [KC, 128, HW], F32, kind="ExternalOutput")

    A_np, E_np = _group_mats()
    A_h = nc.inline_tensor(A_np, name="gn_aggr")
    E_h = nc.inline_tensor(E_np, name="gn_bcast")

    with tile.TileContext(nc) as tc, ExitStack() as ctx:
        per = ctx.enter_context(tc.tile_pool(name="per", bufs=1))
        gwork = ctx.enter_context(tc.tile_pool(name="gwork", bufs=2))
        expp = ctx.enter_context(tc.tile_pool(name="expp", bufs=6))
        recp = ctx.enter_context(tc.tile_pool(name="recp", bufs=2))
        outp = ctx.enter_context(tc.tile_pool(name="outp", bufs=2))

        # ---------- persistent tiles ----------
        XC = [per.tile([128, HW], F32, name=f"XC{i}", tag=f"XC{i}") for i in range(KC)]
        xn = [per.tile([128, HW], BF16, name=f"xn{i}", tag=f"xn{i}") for i in range(KC)]
        wqkT = [per.tile([128, 1024], BF16, name=f"wqkT{i}", tag=f"wqkT{i}") for i in range(KC)]
        wvT = [per.tile([128, 512], BF16, name=f"wvT{i}", tag=f"wvT{i}") for i in range(KC)]
        pwT = [per.tile([128, 512], BF16, name=f"pwT{i}", tag=f"pwT{i}") for i in range(KC)]
        qk = [per.tile([128, HW], BF16, name=f"qk{i}", tag=f"qk{i}") for i in range(8)]
        vt = [per.tile([128, HEADS * (DH + 1)], BF16, name=f"vt{i}", tag=f"vt{i}")
              for i in range(8)]
        att = [per.tile([128, HW], BF16, name=f"att{i}", tag=f"att{i}") for i in range(KC)]

        # consolidated small tensors (one strided DMA each)
        gnwt = per.tile([128, KC], F32, name="gnwt", tag="gnwt")
        gnbt = per.tile([128, KC], F32, name="gnbt", tag="gnbt")
        pbt = per.tile([128, KC], F32, name="pbt", tag="pbt")
        qkbt = per.tile([128, 8], F32, name="qkbt", tag="qkbt")
        AtT = per.tile([128, KC, GROUPS], F32, name="AtT", tag="AtT")
        EtT = per.tile([GROUPS, KC, 128], F32, name="EtT", tag="EtT")
        eps_t = per.tile([GROUPS, 1], F32, name="eps", tag="eps")
        bvb = per.tile([128, 512], F32, name="bvb", tag="bvb")
        ones8 = per.tile([128, HEADS], F32, name="ones8", tag="ones8")
        warm = per.tile([1, 1], F32, name="warm", tag="warm")

        # ---------- input DMAs ----------
        # x on two queues (sync + tensor SWDGE) so groupnorm stats start
        # sooner; q/k weights follow on sync; everything else on gpsimd,
        # highest-urgency first.
        for kc in range(2):
            nc.sync.dma_start(out=XC[kc], in_=x_h[kc * 128:(kc + 1) * 128, :])
        for kc in range(2, KC):
            nc.scalar.dma_start(out=XC[kc], in_=x_h[kc * 128:(kc + 1) * 128, :])
        for kc in range(KC):
            nc.sync.dma_start(out=wqkT[kc], in_=wqkT_h[kc * 128:(kc + 1) * 128, :])
        nc.gpsimd.dma_start(out=AtT, in_=A_h.rearrange("k c g -> c k g"))
        nc.gpsimd.dma_start(out=EtT, in_=E_h.rearrange("k g c -> g k c"))
        nc.gpsimd.dma_start(out=gnwt, in_=gnw_h.rearrange("(k p) -> p k", p=128))
        nc.gpsimd.dma_start(out=gnbt, in_=gnb_h.rearrange("(k p) -> p k", p=128))
        nc.gpsimd.dma_start(out=qkbt, in_=qkvb_h[0:1024].rearrange("(k p) -> p k", p=128))
        for kc in range(KC):
            nc.gpsimd.dma_start(out=wvT[kc], in_=wvT_h[kc * 128:(kc + 1) * 128, :])
        nc.gpsimd.dma_start(out=bvb[:],
                            in_=qkvb_h[1024:1536].unsqueeze(0).partition_broadcast(128))
        nc.gpsimd.dma_start(out=pbt, in_=pb_h.rearrange("(k p) -> p k", p=128))
        for kc in range(KC):
            nc.gpsimd.dma_start(out=pwT[kc], in_=pwT_h[kc * 128:(kc + 1) * 128, :])

        nc.vector.memset(eps_t, EPS)
        nc.vector.memset(ones8, 1.0)
        nc.vector.memset(warm, 1.0)
        # prewarm the SQRT table set while DMAs stream
        nc.scalar.activation(out=warm[:], in_=warm[:], func=AF.Sqrt, scale=1.0)

        # ---------- groupnorm ----------
        with tc.tile_pool(name="ps_gn", bufs=1, space="PSUM") as ps_gn, \
             tc.tile_pool(name="ps_cb", bufs=2, space="PSUM") as ps_cb:
            gstat = ps_gn.tile([GROUPS, 2], F32, name="gstat", tag="gstat")
            me = []
            for kc in range(KC):
                stats = gwork.tile([128, 2, 6], F32, name="stats", tag="stats")
                xv = XC[kc][:].rearrange("p (s f) -> p s f", f=512)
                for s in range(2):
                    nc.vector.bn_stats(out=stats[:, s, :], in_=xv[:, s, :])
                mv = gwork.tile([128, 2], F32, name="mv", tag="mv")
                nc.vector.bn_aggr(out=mv, in_=stats)
                m = gwork.tile([128, 2], F32, name="me", tag="me")
                nc.vector.tensor_mul(out=m[:, 1:2], in0=mv[:, 0:1], in1=mv[:, 0:1])
                nc.vector.tensor_add(out=m[:, 1:2], in0=m[:, 1:2], in1=mv[:, 1:2])
                nc.vector.tensor_copy(out=m[:, 0:1], in_=mv[:, 0:1])
                me.append(m)

            for kc in range(KC):
                nc.tensor.matmul(gstat[:], AtT[:, kc, :],
                                 me[kc][:],
                                 start=(kc == 0), stop=(kc == KC - 1))
            gs = gwork.tile([GROUPS, 2], F32, name="gs", tag="gs")
            nc.vector.tensor_copy(out=gs, in_=gstat)
            var = gwork.tile([GROUPS, 1], F32, name="var", tag="var")
            nc.vector.tensor_mul(out=var, in0=gs[:, 0:1], in1=gs[:, 0:1])
            nc.vector.tensor_sub(out=var, in0=gs[:, 1:2], in1=var)
            srt = gwork.tile([GROUPS, 1], F32, name="srt", tag="srt")
            nc.scalar.activation(out=srt, in_=var, func=AF.Sqrt,
                                 bias=eps_t[:], scale=1.0)
            # prewarm the EXP table set (loads during the qkv phase)
            nc.scalar.activation(out=warm[:], in_=warm[:], func=AF.Exp, scale=1.0)
            gmr = gwork.tile([GROUPS, 2], F32, name="gmr", tag="gmr")
            nc.vector.reciprocal(out=gmr[:, 1:2], in_=srt)
            nc.vector.tensor_copy(out=gmr[:, 0:1], in_=gs[:, 0:1])

            for kc in range(KC):
                cb = ps_cb.tile([128, 2], F32, name="cb", tag="cb")
                nc.tensor.matmul(cb[:], EtT[:, kc, :], gmr[:],
                                 start=True, stop=True)
                cbs = gwork.tile([128, 2], F32, name="cbs", tag="cbs")
                nc.vector.tensor_copy(out=cbs, in_=cb)
                sc = gwork.tile([128, 1], F32, name=f"sc{kc}", tag=f"sc{kc}")
                sh = gwork.tile([128, 1], F32, name=f"sh{kc}", tag=f"sh{kc}")
                nc.vector.tensor_mul(out=sc, in0=cbs[:, 1:2], in1=gnwt[:, kc:kc + 1])
                nc.vector.tensor_mul(out=sh, in0=cbs[:, 0:1], in1=sc)
                nc.vector.tensor_sub(out=sh, in0=gnbt[:, kc:kc + 1], in1=sh)
                nc.vector.tensor_scalar(out=xn[kc][:], in0=XC[kc][:],
                                        scalar1=sc[:], scalar2=sh[:],
                                        op0=OP.mult, op1=OP.add)

        # ---------- qkv / attention / proj ----------
        with tc.tile_pool(name="ps_q", bufs=1, space="PSUM") as ps_q, \
             tc.tile_pool(name="ps_s", bufs=2, space="PSUM") as ps_s, \
             tc.tile_pool(name="ps_av", bufs=3, space="PSUM") as ps_av:

            def emit_qk_half(oc, nh):
                # q/k output chunk oc (128 rows = 2 heads), pixel half nh;
                # yields after each matmul so filler can spread over slots
                pq = ps_q.tile([128, 512], F32, name="pq", tag="pq")
                for kc in range(KC):
                    nc.tensor.matmul(
                        pq[:],
                        wqkT[kc][:, oc * 128:(oc + 1) * 128],
                        xn[kc][:, nh * 512:(nh + 1) * 512],
                        start=(kc == 0), stop=(kc == KC - 1),
                        skip_group_check=True)
                    yield
                nc.vector.tensor_scalar(out=qk[oc][:, nh * 512:(nh + 1) * 512],
                                        in0=pq[:], scalar1=qkbt[:, oc:oc + 1],
                                        scalar2=None, op0=OP.add)
                yield

            def emit_vt(jc):
                pv = ps_q.tile([128, 512], F32, name="pv", tag="pq")
                for kc in range(KC):
                    nc.tensor.matmul(pv[:],
                                     xn[kc][:, jc * 128:(jc + 1) * 128],
                                     wvT[kc][:],
                                     start=(kc == 0), stop=(kc == KC - 1))
                    yield
                vt3 = vt[jc][:].rearrange("p (h e) -> p h e", h=HEADS)
                nc.vector.tensor_copy(out=vt3[:, :, DH:DH + 1],
                                      in_=ones8[:].unsqueeze(-1))
                nc.vector.tensor_add(
                    out=vt3[:, :, 0:DH],
                    in0=pv[:].rearrange("p (h d) -> p h d", h=HEADS),
                    in1=bvb[:].rearrange("p (h d) -> p h d", h=HEADS))
                yield

            def emit_proj_half(oc, nh):
                pp = ps_q.tile([128, 512], F32, name="pp", tag="pq")
                for kc in range(KC):
                    nc.tensor.matmul(
                        pp[:],
                        pwT[kc][:, oc * 128:(oc + 1) * 128],
                        att[kc][:, nh * 512:(nh + 1) * 512],
                        start=(kc == 0), stop=(kc == KC - 1),
                        skip_group_check=True)
                    yield
                ot = outp.tile([128, 512], F32, name="ot", tag="ot")
                nc.vector.scalar_tensor_tensor(
                    out=ot[:], in0=pp[:], scalar=pbt[:, oc:oc + 1],
                    in1=XC[oc][:, nh * 512:(nh + 1) * 512],
                    op0=OP.add, op1=OP.add)
                nc.sync.dma_start(
                    out=y_h[oc * 128:(oc + 1) * 128, nh * 512:(nh + 1) * 512],
                    in_=ot[:])
                yield

            # filler: flat streams of micro-ops (one matmul or one DVE op
            # per step), consumed one per exp slot during attention. The
            # proj nh=0 stream may only run in the LAST window: it reads
            # att[3][:, 0:512], which window (pair 3, half 0) normalizes.
            # deadline order: window (pr,0) needs q[pr]-nh0 and all of
            # k[4+pr] (nh1 from jc=4); window (pr,1) needs q[pr]-nh1.
            def qk_stream():
                for oc, nh in ((1, 0), (5, 0), (5, 1), (1, 1),
                               (2, 0), (6, 0), (6, 1), (2, 1),
                               (3, 0), (7, 0), (7, 1), (3, 1)):
                    yield from emit_qk_half(oc, nh)
            def proj0_stream():
                for oc in range(KC):
                    yield from emit_proj_half(oc, 0)
            fill_qk = qk_stream()
            fill_proj = proj0_stream()

            def emit_filler(gen, n):
                for _ in range(n):
                    try:
                        next(gen)
                    except StopIteration:
                        break

            # upfront: q/k for pair 0 and all v tiles (v feeds the very
            # first AVs; building it as window-0 filler starves the exps)
            for nh in range(2):
                for _ in emit_qk_half(0, nh):
                    pass
                for _ in emit_qk_half(4, nh):
                    pass
            for jc in range(8):
                for _ in emit_vt(jc):
                    pass

            for pr in range(HEADS // 2):
                qt, kt = qk[pr], qk[4 + pr]
                for hf in range(2):
                    last_win = (pr == HEADS // 2 - 1 and hf == 1)
                    pav = [ps_av.tile([DH + 1, 512], F32, name=f"pav{t}",
                                      tag="pav") for t in range(2)]
                    exs = [None] * 8

                    def emit_av(jc, pav=pav, exs=exs, pr=pr):
                        for t in range(2):
                            h = 2 * pr + t
                            nc.tensor.matmul(
                                pav[t][:],
                                vt[jc][:, h * (DH + 1):(h + 1) * (DH + 1)],
                                exs[jc][:, t * 512:(t + 1) * 512],
                                start=(jc == 0), stop=(jc == 7),
                                skip_group_check=True)

                    for jc in range(8):
                        pss = ps_s.tile([128, HW], F32, name="pss", tag="pss")
                        for t in range(2):
                            nc.tensor.matmul(
                                pss[:, t * 512:(t + 1) * 512],
                                kt[64 * t:64 * t + DH, jc * 128:(jc + 1) * 128],
                                qt[64 * t:64 * t + DH,
                                   hf * 512:(hf + 1) * 512],
                                start=True, stop=True)
                        ex = expp.tile([128, HW], BF16, name="expT", tag="expT")
                        nc.scalar.activation(out=ex[:], in_=pss[:],
                                             func=AF.Exp,
                                             scale=float(DH) ** -0.5)
                        exs[jc] = ex
                        if jc > 0:
                            emit_av(jc - 1)
                        if last_win:
                            if jc % 2 == 0:
                                emit_filler(fill_proj, 5)
                        elif jc % 4 == 0:
                            emit_filler(fill_qk, 5)
                    emit_av(7)
                    # softmax normalize: row 64 of each pav is the
                    # denominator. Per-head chains (DVE recip, gpsimd
                    # broadcast, DVE multiply from PSUM) run while the next
                    # window's AVs use a different pav slot.
                    for t in range(2):
                        dn = recp.tile([1, 512], F32, name=f"den{t}",
                                       tag=f"den{t}")
                        nc.vector.tensor_copy(out=dn[:],
                                              in_=pav[t][DH:DH + 1, :])
                        rc = recp.tile([1, 512], F32, name=f"rec{t}",
                                       tag=f"rec{t}")
                        nc.vector.reciprocal_approx_fast(out=rc[:], in_=dn[:])
                        rb = recp.tile([DH, 512], F32, name=f"rb{t}",
                                       tag=f"rb{t}")
                        nc.gpsimd.partition_broadcast(out_ap=rb[:],
                                                      in_ap=rc[:])
                        nc.vector.tensor_mul(
                            out=att[pr][64 * t:64 * t + DH,
                                        hf * 512:(hf + 1) * 512],
                            in0=pav[t][0:DH, :],
                            in1=rb[:])

            if DBG:
                dbp = ctx.enter_context(tc.tile_pool(name="dbp", bufs=2))
                def dump(dst, t):
                    f = dbp.tile(list(t.shape), F32, name="db", tag="db")
                    nc.vector.tensor_copy(out=f[:], in_=t[:])
                    nc.sync.dma_start(out=dst, in_=f[:])
                for kc in range(KC):
                    dump(dbg_xn[kc * 128:(kc + 1) * 128, :], xn[kc])
                    dump(dbg_att[kc], att[kc])
                for oc in range(8):
                    dump(dbg_qk[oc], qk[oc])
                for jc in range(8):
                    dump(dbg_vt[jc], vt[jc])
            # drain any remaining filler, then the final proj half
            emit_filler(fill_qk, 1000)
            emit_filler(fill_proj, 1000)
            for oc in range(KC):
                for _ in emit_proj_half(oc, 1):
                    pass
    nc.compile()
    return nc


_NC = None


def _get_nc():
    global _NC
    if _NC is None:
        _NC = _build()
    return _NC


def _run(inputs, **kwargs):
    nc = _get_nc()
    x = np.ascontiguousarray(np.asarray(inputs["x"], dtype=np.float32))
    qkv_w = np.asarray(inputs["qkv_w"], np.float32)
    proj_w = np.asarray(inputs["proj_w"], np.float32)
    shared = {
        "wqkT": np.ascontiguousarray(qkv_w[0:1024].T).astype(ml_dtypes.bfloat16),
        "wvT": np.ascontiguousarray(qkv_w[1024:1536].T).astype(ml_dtypes.bfloat16),
        "pwT": np.ascontiguousarray(proj_w.T).astype(ml_dtypes.bfloat16),
        "qkv_b": np.ascontiguousarray(np.asarray(inputs["qkv_b"], np.float32)),
        "proj_b": np.ascontiguousarray(np.asarray(inputs["proj_b"], np.float32)),
        "gn_w": np.ascontiguousarray(np.asarray(inputs["gn_w"], np.float32)),
        "gn_b": np.ascontiguousarray(np.asarray(inputs["gn_b"], np.float32)),
    }
    in_maps = [dict(shared, x=x[m].reshape(C, HW)) for m in range(B)]
    res = run_bass_kernel_spmd(nc, in_maps, core_ids=list(range(N_CORES)), **kwargs)
    out = np.stack([res.results[m]["out"] for m in range(B)])
    return out.reshape(B, C, 32, 32).astype(np.float32), res


def kernel(**inputs):
    out, _ = _run(inputs)
    return out
